# revision 8
# baseline (speedup 1.0000x reference)
"""AlignedTripletLoss Trainium2 kernel v3 (8 cores, one-pass ln ACT).

Math (matches reference.py):
  x_hat = x / ||x||_2 per (image, part) row
  c[(a,i),(b,j)] = <x_hat_(a,i), x_hat_(b,j)>;  d = sqrt(2 - 2c)
  t = tanh(0.5*d);  dtw[a,b] = monotone min-path over the 8x8 grid t[i][j]
  ap = max over positives, an = min over negatives, loss = mean(relu(ap-an+0.3))

Design vs v2 (sqrt+tanh two-pass):
 - ONE ACT pass per element: t(u) = tanh(sqrt(u)/2) with u = 2-2c is
   approximated by A*ln(alpha*u + beta) + B (max fit err ~6e-4 in t units,
   tuned end-to-end to ~1e-4 loss err offline).  Every DTW path has exactly
   15 cells and hard-mining commutes with the monotone affine map, so A and
   B fold out on the host: dtw_t = A*dtw_ln + 15B, ap-an = A*(ap_ln-an_ln).
   The ln argument is pre-scaled so outputs center on 0 (the ln-scale shift
   is absorbed into B), which keeps the fp16 wavefront sums small.
   ACT work halves vs v2 and the sqrt<->tanh table thrash (1283ns/switch)
   disappears: only the natural_log table is ever loaded.
 - The hardware Ln table was sweep-verified exact to 6e-8 (fp32 out).
 - DTW wavefront columns are split DVE / Pool(gpsimd) ~72/28 so both
   engines work the min+add pyramid concurrently; per-pyramid deep-slot
   memsets use the read-parity trick (Va even slots, Vb odd slots only).
 - The wavefront of batch b is emitted interleaved with batch b's own
   matmul+ACT stream (step s only needs rsqrt.. i<=min(7,s)), so DVE/Pool
   overlap ACT within a batch, not just across batches.
 - Host pre-normalizes features and ships xrT in its on-chip layout
   [d, j, col] directly (same bytes as v2's xr5a), killing the on-chip
   transpose phase, its affine_select diag prep, and the rn scale fold.
 - Step 14 of each pyramid writes straight into dtwc (fp16), no copy.
 - tmine psum->sbuf copies run on ACT (Copy lives in every table set).
Sharding: same symmetric circulant block cover as v1/v2 (core k owns
blocks k..k+4; transposed mining covers the partner orientation; host
combines per-anchor min/max partials in ln units, then unfolds).
"""

import numpy as np

N, M, D = 1024, 8, 128
MARGIN = 0.3
NCORES = 8
A = N // NCORES          # 128 anchors per core
NDIAG = 5
NCOL = NDIAG * A         # 640 columns per core
CBS = [256, 256, 128]    # column batches (blocks 0,1 | 2,3 | 4)
COL0 = [0, 256, 512]
CDS = [184, 184, 92]     # DVE's column share of each batch (rest on Pool)
CBMAX = 256
NB = len(CBS)
BIG = 30000.0

# t(u) = tanh(sqrt(u)/2) ~= A_LN * ln(AL_LN*u + BE_LN) + B_LN,  u = 2 - 2c
AL_LN = 0.52341955
BE_LN = 0.13862509
A_LN = 0.25257000
# ACT computes ln(ACT_SCALE * c + ACT_BIAS)
ACT_SCALE = -2.0 * AL_LN
ACT_BIAS = 2.0 * AL_LN + BE_LN

_CACHE = {}


def _build_nc():
    import concourse.bacc as bacc
    import concourse.mybir as mybir
    import concourse.tile as tile
    from concourse.tile import add_dep_helper
    from concourse.masks import make_identity

    fp32 = mybir.dt.float32
    fp16 = mybir.dt.float16
    AF = mybir.ActivationFunctionType
    OP = mybir.AluOpType
    AX = mybir.AxisListType

    nc = bacc.Bacc("TRN2", target_bir_lowering=False, debug=False,
                   num_devices=NCORES)

    xr_in = nc.dram_tensor("xrT", [128, M, NCOL], fp16, kind="ExternalInput")
    mop_in = nc.dram_tensor("m_own_pos", [A, NCOL], fp16, kind="ExternalInput")
    mon_in = nc.dram_tensor("m_own_neg", [A, NCOL], fp16, kind="ExternalInput")
    mtp_in = nc.dram_tensor("m_t_pos", [A, (NDIAG - 1) * A], fp16,
                            kind="ExternalInput")
    mtn_in = nc.dram_tensor("m_t_neg", [A, (NDIAG - 1) * A], fp16,
                            kind="ExternalInput")
    out_t = nc.dram_tensor("partials", [A, 10], fp32, kind="ExternalOutput")

    with tile.TileContext(nc) as tc:
        with tc.tile_pool(name="persist", bufs=1) as persist:
            xrT = persist.tile([128, M, NCOL], fp16)   # [d][j][col]
            mop = persist.tile([128, NCOL], fp16)
            mon = persist.tile([128, NCOL], fp16)
            mtp = persist.tile([128, (NDIAG - 1) * A], fp16)
            mtn = persist.tile([128, (NDIAG - 1) * A], fp16)
            Tar = persist.tile([128, 2, 120, CBMAX], fp16)  # slot-major T
            Va = persist.tile([128, 9, CBMAX], fp16)
            Vb = persist.tile([128, 9, CBMAX], fp16)
            dtwc = persist.tile([128, NCOL], fp32)
            ident = persist.tile([128, 128], fp32)
            apacc = persist.tile([128, NB], fp32)
            anacc = persist.tile([128, NB], fp32)
            pout = persist.tile([128, 10], fp32)
            biasT = persist.tile([128, 1], fp32)
            sclT = persist.tile([128, 1], fp32)
            warm = persist.tile([128, 1], fp32)

            # xrT loads: 16 chunks across DMA queues; batch 0 needs
            # cols 0:256 of every j first.
            H = NCOL // 2
            for half in range(2):
                for j in range(M):
                    nc.sync.dma_start(xrT[:, j, half * H:(half + 1) * H],
                                      xr_in[:, j, half * H:(half + 1) * H])
            nc.sync.dma_start(mop[:], mop_in[:])
            nc.sync.dma_start(mon[:], mon_in[:])
            nc.sync.dma_start(mtp[:], mtp_in[:])
            nc.sync.dma_start(mtn[:], mtn_in[:])

            # permanent bottom-boundary pad for the min-plus wavefront
            nc.gpsimd.memset(Va[:, 0:1, :], BIG)
            nc.gpsimd.memset(Vb[:, 0:1, :], BIG)
            make_identity(nc, ident[:])
            nc.gpsimd.memset(biasT[:], ACT_BIAS)
            nc.gpsimd.memset(sclT[:], ACT_SCALE)

            act_chain = [None]

            def act(out, in_, func, **kw):
                # keep the ACT queue in emission order: the scheduler is
                # otherwise free to hoist late-dependency copies ahead of
                # rsqrts, which stalls the whole engine behind a pyramid
                inst = nc.scalar.activation(out, in_, func, **kw)
                if act_chain[0] is not None:
                    add_dep_helper(inst.ins, act_chain[0].ins, sync=False,
                                   reason="ACT queue order")
                act_chain[0] = inst
                return inst

            with (
                tc.tile_pool(name="mpsum", bufs=2, space="PSUM") as mpsum,
                tc.tile_pool(name="mtmp", bufs=4) as mtmp,
            ):
                # hoist the ln table load to t~0
                act(warm[:], biasT[:], AF.Ln)

                TSLOT = [0, 1, 0]

                def emit_batch(nb):
                    CB = CBS[nb]
                    col0 = COL0[nb]
                    CD = CDS[nb]
                    T = Tar[:, TSLOT[nb], :, :]
                    RNG = [(nc.vector, 0, CB)]

                    # deep-slot pads: slot k is read-before-write in buffer
                    # parity (k-2)%2 only -> Va even slots, Vb odd slots.
                    # (Pool: the only tensor ops its ucode supports here are
                    # memset/tcopy/add/sub/mult -- no 2-tensor min/max -- so
                    # the wavefront itself runs on DVE.)
                    nc.gpsimd.memset(Va[:, 2:9:2, 0:CB], BIG)
                    nc.gpsimd.memset(Vb[:, 3:9:2, 0:CB], BIG)

                    def step(s):
                        # buffer(step s) = Va if s even else Vb
                        Vc, Vp = (Va, Vb) if s % 2 == 0 else (Vb, Va)
                        i0, i1 = max(0, s - 7), min(7, s)
                        k0, k1 = i0 + 1, i1 + 2
                        for eng, c0, c1 in RNG:
                            if s == 0:
                                nc.gpsimd.tensor_copy(Va[:, 1:2, c0:c1],
                                                      T[:, 0:1, c0:c1])
                            elif s < 14:
                                eng.tensor_tensor(
                                    Vc[:, k0:k1, c0:c1],
                                    Vp[:, k0 - 1:k1 - 1, c0:c1],
                                    Vp[:, k0:k1, c0:c1], OP.min)
                                eng.tensor_tensor(
                                    Vc[:, k0:k1, c0:c1],
                                    Vc[:, k0:k1, c0:c1],
                                    T[:, 8 * s + i0:8 * s + i1 + 1, c0:c1],
                                    OP.add)
                            else:
                                eng.tensor_tensor(
                                    Vc[:, 8:9, c0:c1], Vp[:, 7:8, c0:c1],
                                    Vp[:, 8:9, c0:c1], OP.min)
                                eng.tensor_tensor(
                                    dtwc[:, col0 + c0:col0 + c1].rearrange(
                                        "p (o c) -> p o c", o=1),
                                    Vc[:, 8:9, c0:c1],
                                    T[:, 119:120, c0:c1], OP.add)

                    for i in range(M):
                        pp = mpsum.tile([128, M, CBMAX], fp32, tag="pp")
                        for jp in range(0, M, 2):
                            nc.tensor.matmul(
                                pp[:, jp:jp + 2, :CB],
                                lhsT=xrT[:, i, 0:128],
                                rhs=xrT[:, jp:jp + 2, col0:col0 + CB],
                                start=True, stop=True)
                        act(T[:, 9 * i:9 * i + 57:8, :CB], pp[:, :, :CB],
                            AF.Ln, scale=sclT[:, 0:1], bias=biasT[:, 0:1])
                        step(i)
                    for s in range(8, 15):
                        step(s)

                    # own-anchor mining over this batch's columns
                    dtw = dtwc[:, col0:col0 + CB]
                    tp = mtmp.tile([128, CBMAX], fp16, tag="tp")
                    nc.vector.tensor_tensor(
                        tp[:, :CB], dtw, mop[:, col0:col0 + CB], OP.add)
                    nc.vector.tensor_reduce(
                        apacc[:, nb:nb + 1], tp[:, :CB], axis=AX.X, op=OP.max)
                    tn = mtmp.tile([128, CBMAX], fp16, tag="tn")
                    nc.vector.tensor_tensor(
                        tn[:, :CB], dtw, mon[:, col0:col0 + CB], OP.add)
                    nc.vector.tensor_reduce(
                        anacc[:, nb:nb + 1], tn[:, :CB], axis=AX.X, op=OP.min)

                def tmine(d):
                    """partner-anchor mining via PE transpose of block d."""
                    ptp = mpsum.tile([128, M, CBMAX], fp32, tag="pp")
                    nc.tensor.transpose(
                        ptp[:, 0, :128], dtwc[:, d * A:(d + 1) * A],
                        ident[:])
                    tb = mtmp.tile([128, 128], fp16, tag="tb")
                    act(tb[:], ptp[:, 0, :128], AF.Copy)
                    tpp = mtmp.tile([128, 128], fp16, tag="tpp")
                    nc.vector.tensor_tensor(
                        tpp[:], tb[:], mtp[:, (d - 1) * A:d * A], OP.add)
                    nc.vector.tensor_reduce(
                        pout[:, 2 * d + 1:2 * d + 2], tpp[:],
                        axis=AX.X, op=OP.max)
                    nc.vector.tensor_tensor(
                        tpp[:], tb[:], mtn[:, (d - 1) * A:d * A], OP.add)
                    nc.vector.tensor_reduce(
                        pout[:, 2 * d:2 * d + 1], tpp[:],
                        axis=AX.X, op=OP.min)

                # tmine PE transposes would block the in-order PE queue on
                # the previous batch's whole pyramid; keep them all at the
                # end where the PE is idle anyway.
                emit_batch(0)
                emit_batch(1)
                emit_batch(2)
                tmine(1)
                tmine(2)
                tmine(3)
                tmine(4)

                nc.vector.tensor_reduce(
                    pout[:, 0:1], anacc[:], axis=AX.X, op=OP.min)
                nc.vector.tensor_reduce(
                    pout[:, 1:2], apacc[:], axis=AX.X, op=OP.max)
                nc.sync.dma_start(out_t[:], pout[:])

    nc.compile()
    return nc


def _get_nc():
    if "nc" not in _CACHE:
        _CACHE["nc"] = _build_nc()
    return _CACHE["nc"]


def kernel(inputs, labels, _trace=False, _trace_cores=None):
    from concourse.bass_utils import run_bass_kernel_spmd

    x = np.asarray(inputs, dtype=np.float32).reshape(N, M, D)
    xn = x / (np.linalg.norm(x, axis=2, keepdims=True) + 1e-12)
    x16 = xn.astype(np.float16)
    lab = np.asarray(labels)

    nc = _get_nc()
    in_maps = []
    for c in range(NCORES):
        blocks = [(c + d) % NCORES for d in range(NDIAG)]
        col_img = np.concatenate([np.arange(b * A, (b + 1) * A)
                                  for b in blocks])
        row_img = np.arange(c * A, (c + 1) * A)
        # xrT[d, j, col] = x16[col_img[col], j, d]
        xrT = np.ascontiguousarray(x16[col_img].transpose(2, 1, 0))
        eq_own = lab[row_img][:, None] == lab[col_img][None, :]
        m_own_pos = np.where(eq_own, np.float16(0.0), np.float16(-BIG))
        m_own_neg = np.where(eq_own, np.float16(BIG), np.float16(0.0))
        mtp_l, mtn_l = [], []
        for d in range(1, NDIAG):
            arow = lab[np.arange(blocks[d] * A, (blocks[d] + 1) * A)]
            eq_t = arow[:, None] == lab[row_img][None, :]
            mtp_l.append(np.where(eq_t, np.float16(0.0), np.float16(-BIG)))
            mtn_l.append(np.where(eq_t, np.float16(BIG), np.float16(0.0)))
        in_maps.append({
            "xrT": xrT,
            "m_own_pos": np.ascontiguousarray(m_own_pos.astype(np.float16)),
            "m_own_neg": np.ascontiguousarray(m_own_neg.astype(np.float16)),
            "m_t_pos": np.ascontiguousarray(
                np.concatenate(mtp_l, axis=1).astype(np.float16)),
            "m_t_neg": np.ascontiguousarray(
                np.concatenate(mtn_l, axis=1).astype(np.float16)),
        })
    res = run_bass_kernel_spmd(
        nc, in_maps, core_ids=list(range(NCORES)), trace=_trace,
        trace_cores=_trace_cores)
    if _trace:
        _CACHE["last_results"] = res

    # combine per-anchor partials in ln units, then unfold the affine map
    an_all = np.full((NCORES, A), np.inf, dtype=np.float32)
    ap_all = np.full((NCORES, A), -np.inf, dtype=np.float32)
    for c in range(NCORES):
        p = res.results[c]["partials"]  # [A, 10]
        for d in range(NDIAG):
            blk = (c + d) % NCORES
            an_all[blk] = np.minimum(an_all[blk], p[:, 2 * d])
            ap_all[blk] = np.maximum(ap_all[blk], p[:, 2 * d + 1])
    loss_vec = np.maximum(
        np.float32(A_LN) * (ap_all.reshape(-1) - an_all.reshape(-1))
        + np.float32(MARGIN), np.float32(0.0))
    return np.asarray(loss_vec.mean(), dtype=np.float32)
